# revision 2
# baseline (speedup 1.0000x reference)
"""CTC loss Bass kernel for Trainium2, 8-core data-parallel.

Algorithm (per core, 128 batch rows on 128 partitions):
  Reference: loss = -logsumexp of CTC alpha recursion over softmax probs
  p~[t,c] = (y[t,c]+eps)/(S_t + C*eps),  S_t = row sum.

  Gauge transform: divide alpha by prod_t (K * p~blank[t]) with K = 1/v,
  v = bf16(exp(-1.2)).  Then the even (blank) states follow
      A_e[t,k] = (A_e[t-1,k] + A_o[t-1,k-1]) * v
  and the odd (label) states follow
      A_o[t,k] = (A_o[t-1,k] + A_e[t-1,k] + sk[k]*A_o[t-1,k-1]) * r[t,k]
  with r[t,k] = v*(y[t,lab_k]+eps)/(y[t,blank]+eps)  -- row sums cancel.
  Both are first-order recurrences along t -> DVE tensor_tensor_scan,
  3 vector ops per label column instead of a 256-step time loop.

  Label/blank/rowsum extraction per batch row is an fp32 PE matmul
  against a host-built one-hot selection matrix: sel[c, 0:64]=v at lab_k,
  sel[127, 64]=1 (blank), sel[:, 65]=1 (row sum), applied to y transposed
  on the host to [B, C, T] (pure layout prep; the device still streams all
  of y).  PSUM [66, 256] per batch row is evacuated by DVE/ACT copies and
  relaid out per-b via SBUF-SBUF DMA into a [b, 66*256] buffer (the scan
  layout).

  loss = -( ln(A_e_fin + A_o_fin) + sum_t ln(yb+eps) - sum_t ln(S+C*eps)
            + T*ln K ).
"""

import numpy as np
import ml_dtypes

import concourse.bacc as bacc
import concourse.bass as bass
import concourse.mybir as mybir
import concourse.tile as tile
from concourse.bass_utils import run_bass_kernel_spmd

N_CORES = 8
B_FULL, T, C, L = 1024, 256, 128, 64
B_LOC = B_FULL // N_CORES
J = L + 1  # 64 label columns + ones(rowsum); blank via direct DMA
EPS = 1e-7
LOGK = 1.2
# v = 1/K folded into the selection matrix so label gathers come out
# pre-scaled.  All gauge bookkeeping uses this exact fp32 value.
V_SCALE = float(np.float32(np.exp(-LOGK)))
LOGK_EFF = float(-np.log(np.float64(V_SCALE)))

GB = 8  # batch rows per staged DMA load

_CACHE: dict = {}


def _build_bass(repeat: int = 1) -> bass.Bass:
    f32 = mybir.dt.float32
    fp16 = mybir.dt.float16
    bf16 = mybir.dt.bfloat16
    i8 = mybir.dt.int8
    nc = bacc.Bacc()

    yt = nc.dram_tensor("y_t", [C, B_LOC, T], bf16, kind="ExternalInput")
    ybl = nc.dram_tensor("y_blank", [B_LOC, T], f32, kind="ExternalInput")
    sel = nc.dram_tensor("sel", [C, B_LOC * J], i8, kind="ExternalInput")
    sk = nc.dram_tensor("sk", [B_LOC, L], f32, kind="ExternalInput")
    loss = nc.dram_tensor("loss", [B_LOC, 1], f32, kind="ExternalOutput")
    # DRAM bounce for the (j <-> b) relayout, bf16 (labels + rowsum rows)
    gs_lab = nc.dram_tensor("gs_lab", [B_LOC, J * T], fp16)

    from contextlib import ExitStack

    with ExitStack() as ctx:
        tc = ctx.enter_context(tile.TileContext(nc))
        singles = ctx.enter_context(tc.tile_pool(name="singles", bufs=1))
        stage = ctx.enter_context(tc.tile_pool(name="stage", bufs=3))
        psp = ctx.enter_context(tc.tile_pool(name="psp", bufs=2, space="PSUM"))
        small = ctx.enter_context(tc.tile_pool(name="small", bufs=1))

        sk_sb = singles.tile([B_LOC, L], f32)
        nc.scalar.dma_start(out=sk_sb, in_=sk[:, :])

        # Gathered values, b-partitioned (the scan layout)
        gbuf_lab = singles.tile([B_LOC, J * T], fp16)
        gblank = singles.tile([B_LOC, T], f32)

        gsl3 = gs_lab.rearrange("b (k t) -> b k t", t=T)

        for _rep in range(repeat):
            # blank column needs no gather (and stays fp32-exact)
            nc.sync.dma_start(out=gblank, in_=ybl[:, :])
            # ---- bulk: load, select+rowsum via fp32 matmul, bf16 bounce ----
            for bg in range(B_LOC // GB):
                yst = stage.tile([C, GB, T], bf16, tag="yst")
                nc.sync.dma_start(out=yst, in_=yt[:, bg * GB:(bg + 1) * GB, :])
                sel_i8 = stage.tile([C, GB * J], i8, tag="sel_i8")
                nc.scalar.dma_start(
                    out=sel_i8, in_=sel[:, bg * GB * J:(bg + 1) * GB * J])
                sel_sb = stage.tile([C, GB * J], bf16, tag="sel_sb")
                nc.scalar.copy(out=sel_sb, in_=sel_i8)
                ps8 = psp.tile([J, GB, T], f32, tag="ps8")
                for j in range(GB):
                    nc.tensor.matmul(
                        ps8[:, j, :], sel_sb[:, j * J:(j + 1) * J],
                        yst[:, j, :], start=True, stop=True,
                    )
                stg = stage.tile([J, GB, T], fp16, tag="stg")
                if bg % 2 == 0:
                    nc.vector.tensor_copy(stg, ps8)
                else:
                    nc.scalar.copy(out=stg, in_=ps8)
                bsl = slice(bg * GB, (bg + 1) * GB)
                nc.sync.dma_start(
                    out=gsl3[bsl, :, :].rearrange("b k t -> k b t"), in_=stg)

            # All-engine barrier: absorbs the bulk phase's cross-engine deps so
            # the scan-phase TensorScalarPtr (STT/scan) instructions carry no
            # semaphore waits (the S2S2D2_STT encoding has no room for them).
            tc.strict_bb_all_engine_barrier()

            # ---- per-(b,t) scalars: recip of blank, ln sums ----
            s_ap = gbuf_lab[:, L * T:J * T]   # row sums (bf16) [128, T]

            eps_t = small.tile([B_LOC, 1], f32)
            nc.vector.memset(eps_t, EPS)
            ceps_t = small.tile([B_LOC, 1], f32)
            nc.vector.memset(ceps_t, C * EPS)
            zero_t = small.tile([B_LOC, 1], f32)
            nc.vector.memset(zero_t, 0.0)
            # First DVE touch of DMA-written tiles: multi-wait-capable copy
            # (TensorScalarPtr can hold at most one semaphore wait).
            dve_sync = small.tile([B_LOC, 1], f32)
            nc.vector.tensor_copy(dve_sync, gblank[:, 0:1])
            tmp = small.tile([B_LOC, T], f32)
            nc.vector.tensor_scalar_add(tmp, gblank, EPS)
            recip = small.tile([B_LOC, T], f32)
            nc.vector.reciprocal(recip, tmp)
            # fold the gauge scale v into the reciprocal: recip = v/(yb+eps)
            nc.vector.tensor_scalar_mul(recip, recip, V_SCALE)

            # r[b, k*T + t] = (y_lab + eps) * v / (yb + eps)
            # k-chunked loads + STTs so they pipeline with the scan chain
            rbuf = singles.tile([B_LOC, L * T], f32)
            KC = 8
            for kc in range(L // KC):
                ksl = slice(kc * KC * T, (kc + 1) * KC * T)
                nc.sync.dma_start(out=gbuf_lab[:, ksl], in_=gs_lab[:, ksl])
                recip_b = bass.AP(
                    tensor=recip.tensor, offset=recip.offset,
                    ap=[list(recip.ap[0]), [0, KC], list(recip.ap[1])],
                )
                nc.vector.scalar_tensor_tensor(
                    out=rbuf[:, ksl].rearrange("p (k t) -> p k t", t=T),
                    in0=gbuf_lab[:, ksl].rearrange("p (k t) -> p k t", t=T),
                    scalar=EPS, in1=recip_b,
                    op0=mybir.AluOpType.add, op1=mybir.AluOpType.mult,
                )
            # rowsum rows arrive with the last chunk
            nc.scalar.dma_start(
                out=gbuf_lab[:, L * T:J * T], in_=gs_lab[:, L * T:J * T])

            lnyb = small.tile([B_LOC, T], f32)
            lnyb_acc = small.tile([B_LOC, 1], f32)
            nc.scalar.activation(
                out=lnyb, in_=gblank, func=mybir.ActivationFunctionType.Ln,
                bias=eps_t[:, 0:1], scale=1.0, accum_out=lnyb_acc,
            )
            lnS = small.tile([B_LOC, T], f32)
            lnS_acc = small.tile([B_LOC, 1], f32)
            nc.scalar.activation(
                out=lnS, in_=s_ap, func=mybir.ActivationFunctionType.Ln,
                bias=ceps_t[:, 0:1], scale=1.0, accum_out=lnS_acc,
            )

            # ---- scan phase ----
            invk_col = small.tile([B_LOC, T], f32)
            nc.vector.memset(invk_col, V_SCALE)
            a_e = small.tile([B_LOC, T + 1], f32)
            nc.vector.memset(a_e[:, 0:1], 0.0)
            zbuf = small.tile([B_LOC, T + 1], f32)
            nc.vector.memset(zbuf, 0.0)
            a_o = [small.tile([B_LOC, T + 1], f32, name=f"ao{i}", tag=f"ao{i}")
                   for i in range(2)]
            nc.vector.memset(a_o[0][:, 0:1], 0.0)
            nc.vector.memset(a_o[1][:, 0:1], 0.0)
            u = small.tile([B_LOC, T], f32)

            add = mybir.AluOpType.add
            mult = mybir.AluOpType.mult
            for k in range(L + 1):
                prev = zbuf if k == 0 else a_o[(k - 1) % 2]
                init = 1.0 if k == 0 else 0.0
                nc.vector.tensor_tensor_scan(
                    out=a_e[:, 1:T + 1], data0=prev[:, 0:T],
                    data1=invk_col[:, 0:T], initial=init, op0=add, op1=mult,
                )
                if k == L:
                    break
                nc.vector.scalar_tensor_tensor(
                    out=u, in0=prev[:, 0:T], scalar=sk_sb[:, k:k + 1],
                    in1=a_e[:, 0:T], op0=mult, op1=add,
                )
                nc.vector.tensor_tensor_scan(
                    out=a_o[k % 2][:, 1:T + 1], data0=u,
                    data1=rbuf[:, k * T:(k + 1) * T],
                    initial=init, op0=add, op1=mult,
                )

            # ---- final assembly ----
            fin = small.tile([B_LOC, 1], f32)
            nc.vector.tensor_add(
                fin, a_e[:, T:T + 1], a_o[(L - 1) % 2][:, T:T + 1])
            # ln(fin) via exponent/mantissa split: the ACT Ln LUT is inaccurate
            # below ~1e-20, and fin spans down to ~e^-70.
            i32 = mybir.dt.int32
            fin_i = fin.bitcast(i32)
            ebits = small.tile([B_LOC, 1], i32)
            nc.vector.tensor_scalar(
                out=ebits, in0=fin_i, scalar1=23, scalar2=None,
                op0=mybir.AluOpType.logical_shift_right,
            )
            e_f = small.tile([B_LOC, 1], f32)
            nc.vector.tensor_copy(e_f, ebits)
            mbits = small.tile([B_LOC, 1], i32)
            nc.vector.tensor_scalar(
                out=mbits, in0=fin_i, scalar1=0x7FFFFF, scalar2=(127 << 23),
                op0=mybir.AluOpType.bitwise_and, op1=mybir.AluOpType.bitwise_or,
            )
            lnm = small.tile([B_LOC, 1], f32)
            nc.scalar.activation(
                out=lnm, in_=mbits.bitcast(f32),
                func=mybir.ActivationFunctionType.Ln,
                bias=zero_t[:, 0:1], scale=1.0,
            )
            lnfin = small.tile([B_LOC, 1], f32)
            nc.vector.scalar_tensor_tensor(
                out=lnfin, in0=e_f, scalar=float(np.log(2.0)), in1=lnm,
                op0=mult, op1=add,
            )
            t1 = small.tile([B_LOC, 1], f32)
            nc.vector.tensor_add(t1, lnfin, lnyb_acc)
            t2 = small.tile([B_LOC, 1], f32)
            nc.vector.tensor_sub(t2, t1, lnS_acc)
            loss_t = small.tile([B_LOC, 1], f32)
            nc.scalar.activation(
                out=loss_t, in_=t2, func=mybir.ActivationFunctionType.Copy,
                bias=float(127.0 * np.log(2.0) - T * LOGK_EFF), scale=-1.0,
            )
            nc.scalar.dma_start(out=loss[:, :], in_=loss_t)

    nc.compile()
    return nc


def _host_prep(y_true: np.ndarray):
    lab = y_true.astype(np.int64)
    B = lab.shape[0]
    b_loc = B // N_CORES
    sel = np.zeros((N_CORES, C, b_loc, J), dtype=np.int8)
    core_idx = np.arange(B) // b_loc
    bloc_idx = np.arange(B) % b_loc
    for k in range(L):
        sel[core_idx, lab[:, k], bloc_idx, k] = 1
    sel[:, :, :, J - 1] = 1.0
    sk = np.zeros((B, L), np.float32)
    sk[:, 1:] = (lab[:, 1:] != lab[:, :-1]).astype(np.float32)
    return sel, sk


def _make_in_maps(y_true: np.ndarray, y_pred: np.ndarray) -> list:
    B = y_pred.shape[0]
    b_loc = B // N_CORES
    sel, sk = _host_prep(y_true)
    in_maps = []
    for i in range(N_CORES):
        in_maps.append({
            "y_t": np.ascontiguousarray(
                y_pred[i * b_loc:(i + 1) * b_loc].transpose(2, 0, 1)
            ).astype(ml_dtypes.bfloat16),
            "y_blank": np.ascontiguousarray(
                y_pred[i * b_loc:(i + 1) * b_loc, :, C - 1]
            ).astype(np.float32, copy=False),
            "sel": np.ascontiguousarray(sel[i].reshape(C, b_loc * J)),
            "sk": np.ascontiguousarray(sk[i * b_loc:(i + 1) * b_loc]),
        })
    return in_maps


def kernel(y_true: np.ndarray, y_pred: np.ndarray) -> np.ndarray:
    if "nc" not in _CACHE:
        _CACHE["nc"] = _build_bass()
    nc = _CACHE["nc"]
    in_maps = _make_in_maps(y_true, y_pred)
    res = run_bass_kernel_spmd(nc, in_maps, core_ids=list(range(N_CORES)))
    out = np.concatenate([res.results[i]["loss"] for i in range(N_CORES)], axis=0)
    return out.astype(np.float32, copy=False)



# revision 18
# speedup vs baseline: 1.2535x; 1.2535x over previous
"""CTC loss Bass kernel for Trainium2, 8-core data-parallel. v2.

Per core: 128 batch rows on 128 partitions, T=256, C=128, L=64.

Structure:
  1. Gather phase (PE): per batch-group of 8 rows, matmul one-hot sel
     [C, 65] against y^T [C, T] -> PSUM [65, 8, 256] f32 holding
     (labels 0..63, rowsum) per row. Scattered straight from PSUM into
     gbuf [b, 65*T] f32 via DMA; backward-half label columns (32..63)
     are stored time-reversed.
  2. Alpha chain (DVE): c=1 gauge -- odd multipliers are the raw bf16
     label probs, even multiplier is y_blank + eps. Forward DP covers
     label columns 0..31, backward (suffix) DP covers 63..32; the two
     independent chains are interleaved op-by-op on the DVE so every
     scan runs at the independent-op rate (drain hidden). 3 ops per
     column: scan_E / STT(u or drive) / scan_O.
  3. Cut combine: P = sum_t O[31][t] * drive31[t+1] (STT with accum),
     loss = lnS_acc - ln(P), lnS_acc = sum_t ln(rowsum_t + C*eps) on ACT.

Empirical range (uniform y): ln P in [12, 32]; fwd states < e^44,
f32 throughout the chain; gather values exact bf16 in f32.
"""

import numpy as np
import ml_dtypes

import concourse.bacc as bacc
import concourse.bass as bass
import concourse.mybir as mybir
import concourse.tile as tile
from concourse.bass_utils import run_bass_kernel_spmd

N_CORES = 8
B_FULL, T, C, L = 1024, 256, 128, 64
B_LOC = B_FULL // N_CORES
J = L + 1  # 64 label columns + rowsum column
K1 = 31    # forward chain covers odd columns 0..K1; backward L-1..K1+1
EPS = 1e-7
GB = 8     # batch rows per staged matmul group

_CACHE: dict = {}


def _rev_last(ap: bass.AP) -> bass.AP:
    """Reverse the last non-degenerate free dim of an AP; drop [*,1] dims
    beyond the partition dim (they break DMA AP balancing)."""
    dims = [list(d) for d in ap.ap]
    dims = [dims[0]] + [d for d in dims[1:] if d[1] != 1]
    stride, count = dims[-1]
    dims[-1] = [-stride, count]
    return bass.AP(
        tensor=ap.tensor,
        offset=ap.offset + (count - 1) * stride,
        ap=dims,
    )


def _build_bass() -> bass.Bass:
    f32 = mybir.dt.float32
    bf16 = mybir.dt.bfloat16
    i8 = mybir.dt.int8
    nc = bacc.Bacc()
    add = mybir.AluOpType.add
    mult = mybir.AluOpType.mult

    yt = nc.dram_tensor("y_t", [C, B_LOC, T], bf16, kind="ExternalInput")
    ybl = nc.dram_tensor("y_blank", [B_LOC, T], f32, kind="ExternalInput")
    yblr = nc.dram_tensor("y_blank_r", [B_LOC, T], f32, kind="ExternalInput")
    sel = nc.dram_tensor("sel", [C, B_LOC * J], i8, kind="ExternalInput")
    sk = nc.dram_tensor("sk", [B_LOC, L], f32, kind="ExternalInput")
    loss = nc.dram_tensor("loss", [B_LOC, 1], f32, kind="ExternalOutput")
    # DRAM bounce for the (j <-> b) relayout (SBUF->SBUF partition
    # scatter is rejected by the BIR verifier; DRAM dst is legal)
    gs = nc.dram_tensor("gs", [B_LOC, J * T], f32)

    from contextlib import ExitStack

    with ExitStack() as ctx:
        tc = ctx.enter_context(tile.TileContext(nc))
        singles = ctx.enter_context(tc.tile_pool(name="singles", bufs=1))
        stage = ctx.enter_context(tc.tile_pool(name="stage", bufs=3))
        psp = ctx.enter_context(tc.tile_pool(name="psp", bufs=2, space="PSUM"))
        small = ctx.enter_context(tc.tile_pool(name="small", bufs=1))

        sk_sb = singles.tile([B_LOC, L], f32)
        nc.scalar.dma_start(out=sk_sb, in_=sk[:, :])
        ybl_sb = singles.tile([B_LOC, T], f32)
        nc.scalar.dma_start(out=ybl_sb, in_=ybl[:, :])

        # gathered label probs + rowsums, b-partitioned; cols 32..63 reversed
        gbuf = singles.tile([B_LOC, J * T], f32)
        gb3 = gbuf[:, :].rearrange("p (k t) -> p k t", t=T)
        gs3 = gs.rearrange("b (k t) -> b k t", t=T)

        # ---- bulk gather ----
        for bg in range(B_LOC // GB):
            yst = stage.tile([C, GB, T], bf16, tag="yst")
            nc.sync.dma_start(out=yst, in_=yt[:, bg * GB:(bg + 1) * GB, :])
            sel_i8 = stage.tile([C, GB * J], i8, tag="sel_i8")
            nc.scalar.dma_start(
                out=sel_i8, in_=sel[:, bg * GB * J:(bg + 1) * GB * J])
            sel_sb = stage.tile([C, GB * J], bf16, tag="sel_sb")
            nc.scalar.copy(out=sel_sb, in_=sel_i8)
            ps8 = psp.tile([J, GB, T], f32, tag="ps8")
            for j in range(GB):
                nc.tensor.matmul(
                    ps8[:, j, :], sel_sb[:, j * J:(j + 1) * J],
                    yst[:, j, :], start=True, stop=True,
                )
            # evacuate PSUM -> SBUF f32 (DMA cannot read PSUM); alternate
            # engines so neither ScalarE nor DVE bounds the bulk phase
            stg = stage.tile([J, GB, T], f32, tag="stg")
            if bg % 2 == 0:
                nc.vector.tensor_copy(stg, ps8)
            else:
                nc.scalar.copy(out=stg, in_=ps8)
            bsl = slice(bg * GB, (bg + 1) * GB)
            # bounce out k-major to DRAM; backward cols are read
            # time-reversed by the chain via negative-stride APs
            nc.scalar.dma_start(
                out=gs3[bsl, :, :].rearrange("b k t -> k b t"),
                in_=stg[0:J, :, :])

        # reload the b-partitioned gather buffer in k-chunks (contiguous
        # per partition) so the loads pipeline behind the bounce writes
        for kc in range(4):
            ksl = slice(kc * (J * T) // 4, (kc + 1) * (J * T) // 4)
            nc.sync.dma_start(out=gbuf[:, ksl], in_=gs[:, ksl])

        # ---- chain setup (pre-barrier ops independent of gather are fine,
        # but everything below the barrier carries no semaphores) ----
        e_fwd = singles.tile([B_LOC, T], f32)
        nc.vector.tensor_scalar_add(e_fwd, ybl_sb, EPS)
        yblr_sb = singles.tile([B_LOC, T], f32)
        nc.scalar.dma_start(out=yblr_sb, in_=yblr[:, :])
        e_rev = singles.tile([B_LOC, T], f32)
        nc.vector.tensor_scalar_add(e_rev, yblr_sb, EPS)

        TP = T + 1
        zbuf = small.tile([B_LOC, TP], f32)
        nc.vector.memset(zbuf, 0.0)
        Ebuf = small.tile([B_LOC, TP], f32)
        nc.vector.memset(Ebuf, 0.0)
        ubuf = small.tile([B_LOC, TP], f32)
        nc.vector.memset(ubuf, 0.0)
        Obuf = [small.tile([B_LOC, TP], f32, name=f"o{i}", tag=f"o{i}")
                for i in range(2)]
        nc.vector.memset(Obuf[0], 0.0)
        nc.vector.memset(Obuf[1], 0.0)
        dbuf = small.tile([B_LOC, TP], f32)
        nc.vector.memset(dbuf, 0.0)
        bEbuf = [small.tile([B_LOC, TP], f32, name=f"be{i}", tag=f"be{i}")
                 for i in range(2)]
        bObuf = [small.tile([B_LOC, TP], f32, name=f"bo{i}", tag=f"bo{i}")
                 for i in range(2)]
        nc.vector.memset(bEbuf[0], 0.0)
        nc.vector.memset(bEbuf[1], 0.0)
        nc.vector.memset(bObuf[0], 0.0)
        nc.vector.memset(bObuf[1], 0.0)
        pdot = small.tile([B_LOC, TP], f32)
        pacc = small.tile([B_LOC, 1], f32)
        lnS = small.tile([B_LOC, T], f32)
        lnS_acc = small.tile([B_LOC, 1], f32)
        ceps_t = small.tile([B_LOC, 1], f32)
        nc.vector.memset(ceps_t, C * EPS)
        zero_t = small.tile([B_LOC, 1], f32)
        nc.vector.memset(zero_t, 0.0)
        lnP = small.tile([B_LOC, 1], f32)
        loss_t = small.tile([B_LOC, 1], f32)

        tc.strict_bb_all_engine_barrier()

        # post-barrier init copies (depend on gather output / e_fwd)
        nc.vector.tensor_copy(Ebuf[:, 1:2], e_fwd[:, 0:1])
        nc.vector.tensor_copy(Obuf[0][:, 1:2], gb3[:, 0, 0:1])

        # lnS on ScalarE, parallel with the chain
        nc.scalar.activation(
            out=lnS, in_=gb3[:, L, :], func=mybir.ActivationFunctionType.Ln,
            bias=ceps_t[:, 0:1], scale=1.0, accum_out=lnS_acc,
        )

        # ---- the two chains, interleaved ----
        fwd_ops = []
        bwd_ops = []

        def scan(out, d0, d1, init):
            nc.vector.tensor_tensor_scan(
                out=out, data0=d0, data1=d1, initial=init, op0=add, op1=mult)

        # forward: k = 0..K1
        for k in range(K1 + 1):
            Ocur = Obuf[k % 2]
            Oprev = Obuf[(k - 1) % 2] if k > 0 else zbuf
            if k == 0:
                fwd_ops.append(lambda: scan(
                    Ebuf[:, 2:TP], zbuf[:, 1:T], e_fwd[:, 1:T],
                    e_fwd[:, 0:1]))
                fwd_ops.append(lambda: scan(
                    Obuf[0][:, 2:TP], Ebuf[:, 1:T], gb3[:, 0, 1:T],
                    gb3[:, 0, 0:1]))
            else:
                fwd_ops.append(lambda k=k, Oprev=Oprev: scan(
                    Ebuf[:, 2:TP], Oprev[:, 1:T], e_fwd[:, 1:T], 0.0))
                if k == 1:
                    # E[0] must read as 0 for k>=1 (k=0 wrote e[0] there);
                    # zero it after scan_o[0] consumed it, before stt[1]
                    fwd_ops.append(lambda: nc.vector.memset(Ebuf[:, 1:2], 0.0))
                fwd_ops.append(lambda k=k, Oprev=Oprev:
                               nc.vector.scalar_tensor_tensor(
                                   out=ubuf[:, 1:T], in0=Oprev[:, 1:T],
                                   scalar=sk_sb[:, k:k + 1], in1=Ebuf[:, 1:T],
                                   op0=mult, op1=add))
                if k == 2:
                    # O[0][0]=m00 consumed by k=1; zero before O0 reuse at k=2
                    fwd_ops.append(
                        lambda: nc.vector.memset(Obuf[0][:, 1:2], 0.0))
                fwd_ops.append(lambda k=k, Ocur=Ocur: scan(
                    Ocur[:, 2:TP], ubuf[:, 1:T], gb3[:, k, 1:T], 0.0))

        # backward: b_E[64] then k = L-1..K1+1
        bwd_ops.append(lambda: scan(
            bEbuf[0][:, 1:TP], zbuf[:, 0:T], e_rev[:, 0:T], 1.0))
        bEp, bOp = bEbuf[0], bObuf[0]  # bObuf[0] stays all-zero as b_O[64]
        for k in range(L - 1, K1, -1):
            bEc = bEbuf[(L - k) % 2]
            bOc = bObuf[(L - k) % 2]
            bEprev, bOprev = bEp, bOp
            if k == L - 1:
                # drive = b_E[64] (sk_ext[64]=0): skip the STT
                bwd_ops.append(lambda bOc=bOc, bEprev=bEprev, k=k: scan(
                    bOc[:, 1:TP], bEprev[:, 0:T],
                    _rev_last(gb3[:, k, 0:T]), 1.0))
            else:
                bwd_ops.append(lambda bEprev=bEprev, bOprev=bOprev, k=k:
                               nc.vector.scalar_tensor_tensor(
                                   out=dbuf[:, 1:T], in0=bOprev[:, 1:T],
                                   scalar=sk_sb[:, k + 1:k + 2],
                                   in1=bEprev[:, 1:T], op0=mult, op1=add))
                bwd_ops.append(lambda bOc=bOc, k=k: scan(
                    bOc[:, 1:TP], dbuf[:, 0:T],
                    _rev_last(gb3[:, k, 0:T]), 1.0))
            bwd_ops.append(lambda bEc=bEc, bOc=bOc: scan(
                bEc[:, 1:TP], bOc[:, 0:T], e_rev[:, 0:T], 1.0))
            bEp, bOp = bEc, bOc
        # final drive for column K1+1 -> dbuf[1:T] = drive31_rev[0:T-1]
        bwd_ops.append(lambda bEp=bEp, bOp=bOp:
                       nc.vector.scalar_tensor_tensor(
                           out=dbuf[:, 1:T], in0=bOp[:, 1:T],
                           scalar=sk_sb[:, K1 + 1:K1 + 2], in1=bEp[:, 1:T],
                           op0=mult, op1=add))

        # interleave
        na, nb = len(fwd_ops), len(bwd_ops)
        for i in range(max(na, nb)):
            if i < na:
                fwd_ops[i]()
            if i < nb:
                bwd_ops[i]()

        # cut combine: P = sum_t O[K1][t] * drive31[t+1]
        #   O[K1][t] at ObufK1[:, 1+t], t=0..T-2 -> [:, 1:T]
        #   drive31[t+1] = drive31_rev[T-2-t] at dbuf[:, T-1-t] -> reversed AP
        OK1 = Obuf[K1 % 2]
        din = bass.AP(
            tensor=dbuf.tensor, offset=dbuf.offset + (T - 1),
            ap=[list(dbuf.ap[0]), [-1, T - 1]],
        )
        nc.vector.scalar_tensor_tensor(
            out=pdot[:, 1:T], in0=OK1[:, 1:T], scalar=1.0, in1=din,
            op0=mult, op1=mult, accum_out=pacc,
        )
        nc.scalar.activation(
            out=lnP, in_=pacc, func=mybir.ActivationFunctionType.Ln,
            bias=zero_t[:, 0:1], scale=1.0,
        )
        nc.vector.tensor_sub(loss_t, lnS_acc, lnP)
        nc.scalar.dma_start(out=loss[:, :], in_=loss_t)

    nc.compile()
    return nc


def _host_prep(y_true: np.ndarray):
    lab = y_true.astype(np.int64)
    B = lab.shape[0]
    b_loc = B // N_CORES
    sel = np.zeros((N_CORES, C, b_loc, J), dtype=np.int8)
    core_idx = np.arange(B) // b_loc
    bloc_idx = np.arange(B) % b_loc
    for k in range(L):
        sel[core_idx, lab[:, k], bloc_idx, k] = 1
    sel[:, :, :, J - 1] = 1
    sk = np.zeros((B, L), np.float32)
    sk[:, 1:] = (lab[:, 1:] != lab[:, :-1]).astype(np.float32)
    return sel, sk


def _make_in_maps(y_true: np.ndarray, y_pred: np.ndarray) -> list:
    B = y_pred.shape[0]
    b_loc = B // N_CORES
    sel, sk = _host_prep(y_true)
    in_maps = []
    for i in range(N_CORES):
        in_maps.append({
            "y_t": np.ascontiguousarray(
                y_pred[i * b_loc:(i + 1) * b_loc].transpose(2, 0, 1)
            ).astype(ml_dtypes.bfloat16),
            "y_blank": np.ascontiguousarray(
                y_pred[i * b_loc:(i + 1) * b_loc, :, C - 1]
            ).astype(np.float32, copy=False),
            "y_blank_r": np.ascontiguousarray(
                y_pred[i * b_loc:(i + 1) * b_loc, ::-1, C - 1]
            ).astype(np.float32, copy=False),
            "sel": np.ascontiguousarray(sel[i].reshape(C, b_loc * J)),
            "sk": np.ascontiguousarray(sk[i * b_loc:(i + 1) * b_loc]),
        })
    return in_maps


def kernel(y_true: np.ndarray, y_pred: np.ndarray) -> np.ndarray:
    if "nc" not in _CACHE:
        _CACHE["nc"] = _build_bass()
    nc = _CACHE["nc"]
    in_maps = _make_in_maps(y_true, y_pred)
    res = run_bass_kernel_spmd(nc, in_maps, core_ids=list(range(N_CORES)))
    out = np.concatenate([res.results[i]["loss"] for i in range(N_CORES)], axis=0)
    return out.astype(np.float32, copy=False)
